# revision 1
# baseline (speedup 1.0000x reference)
"""GCN (3-layer DemandGNN) Trainium2 Bass kernel — 8-core SPMD.

Node-sharded GCN: each core owns 12500 destination nodes. Per layer:
 - transform activations locally on PE (x@W1 / act@W2), pre-scale by
   dinv (symmetric norm, source side), cast fp16;
 - AllGather the fp16 feature table into each core's DRAM;
 - gather per-edge source rows with batched SWDGE dma_gather: 64-byte
   descriptors over a 256B-strided quad view of the table, 4 residue
   streams (table row % 4) selected by call byte-offset;
 - DVE tree-reduce padded per-dst slot groups into 4 per-stream partials;
 - align partials (each stream has its own dst order) with a small
   combine gather, then dinv (dst side) + bias + relu;
 - layer 3 aggregates act2*dinv and applies W3 afterwards (A(hW)=(Ah)W).

All structure (tile slot classes, call layout) is shared across cores so
one SPMD program serves all 8; only index/feature data differ per core.
"""
import numpy as np

P = 128
F = 32            # hidden feature dim
FIN = 128         # input feature dim
NCORES = 8
N_NODES = 100000

CALL_COLS = 8     # slot columns per dma_gather call (idxs = 128*cols <= ring)
GB_COLS = 240     # gather buffer columns per half (double buffered)
LADDER = [2, 4, 6, 8, 10, 12, 14, 16, 18, 20, 22, 24, 26, 28, 30, 32,
          36, 40, 44, 48, 56, 64, 80, 96, 128, 160, 192, 240]
LAYERS = 3


def _ladder_round(x):
    for v in LADDER:
        if x <= v:
            return v
    raise ValueError(f"slot count {x} exceeds ladder")


class Struct(dict):
    __getattr__ = dict.__getitem__
    __setattr__ = dict.__setitem__


# ======================= host preprocessing =======================

def prep(edge_index, n_nodes=N_NODES, ncores=NCORES):
    own = n_nodes // ncores
    nt = (own + P - 1) // P
    nloc = nt * P
    nloct = nloc + 4                  # +4 null rows (cover all residues)
    ntot = ncores * nloct
    null_quad = nloc // 4             # core0 rows nloc..nloc+3 (zeros)

    src = np.concatenate([np.asarray(edge_index[0], dtype=np.int64),
                          np.arange(n_nodes, dtype=np.int64)])
    dst = np.concatenate([np.asarray(edge_index[1], dtype=np.int64),
                          np.arange(n_nodes, dtype=np.int64)])
    deg = np.bincount(dst, minlength=n_nodes).astype(np.int64)
    dinv = (1.0 / np.sqrt(deg.astype(np.float64))).astype(np.float32)

    core_of_edge = dst // own

    # stream-0 dst order: by total degree (residue-independent to avoid
    # circularity — table rows depend on it, residues depend on table rows)
    s0_orders = []
    s0pos_row = np.zeros(n_nodes, dtype=np.int64)   # p-major row within core
    for c in range(ncores):
        cnt = np.bincount(dst[core_of_edge == c] - c * own, minlength=own)
        order0 = np.argsort(-cnt, kind="stable")
        s0_orders.append(order0)
        pos = np.zeros(own, dtype=np.int64)
        pos[order0] = np.arange(own)
        s0pos_row[c * own:(c + 1) * own] = (pos % P) * nt + (pos // P)

    trow = (np.arange(n_nodes) // own) * nloct + s0pos_row
    vres = trow % 4
    vquad = trow // 4
    assert vquad.max() < 32768

    per_core = []
    need = np.zeros((ncores, 4, nt), dtype=np.int64)
    for c in range(ncores):
        m = core_of_edge == c
        e_src, e_dst_l = src[m], dst[m] - c * own
        streams = []
        for q in range(4):
            mq = vres[e_src] == q
            cq = np.bincount(e_dst_l[mq], minlength=own)
            if q == 0:
                order_q = s0_orders[c]
            else:
                order_q = np.argsort(-cq, kind="stable")
            cq_s = np.concatenate([cq[order_q], np.zeros(nloc - own, np.int64)])
            need[c, q] = cq_s.reshape(nt, P).max(axis=1)
            oq = np.argsort(e_dst_l[mq], kind="stable")
            streams.append(dict(order=order_q,
                                eq_quad=vquad[e_src[mq]][oq],
                                st=np.concatenate([[0], np.cumsum(cq)]),
                                cnt=cq))
        per_core.append(streams)

    tiles_s = np.zeros((4, nt), dtype=np.int64)
    for q in range(4):
        for t in range(nt):
            tiles_s[q, t] = _ladder_round(max(2, int(need[:, q, t].max())))

    # sub-batches: runs of equal s (per stream), tile-aligned, <= GB_COLS
    subbatches = []
    for q in range(4):
        t = 0
        while t < nt:
            s = int(tiles_s[q, t])
            assert s <= GB_COLS
            t1 = t
            while t1 < nt and int(tiles_s[q, t1]) == s and (t1 - t + 1) * s <= GB_COLS:
                t1 += 1
            calls = []
            cols = (t1 - t) * s
            off = 0
            while off < cols:
                k = min(CALL_COLS, cols - off)
                calls.append((off, k))
                off += k
            subbatches.append(Struct(q=q, t0=t, ntiles=t1 - t, s=s, calls=calls))
            t = t1
    total_cols = sum(sb.ntiles * sb.s for sb in subbatches)

    # combine calls (common): 3 partials x nt columns
    cmb_calls = []
    for qi in range(3):
        off = 0
        while off < nt:
            k = min(CALL_COLS, nt - off)
            cmb_calls.append((qi, off, k))
            off += k

    # ---- per-core gather idx (wrapped per call, concatenated) ----
    gidx_cores, cidx_cores, dinv_cores = [], [], []
    for c in range(ncores):
        parts = []
        for sb in subbatches:
            stq = per_core[c][sb.q]
            order_q, s = stq["order"], sb.s
            tc = np.full((sb.ntiles * s, P), null_quad, dtype=np.int16)
            for ti in range(sb.ntiles):
                for p in range(P):
                    gi = (sb.t0 + ti) * P + p
                    if gi >= own:
                        continue
                    d = order_q[gi]
                    k = int(stq["cnt"][d])
                    if k:
                        tc[ti * s:ti * s + k, p] = stq["eq_quad"][stq["st"][d]:stq["st"][d] + k]
            parts.append(tc)
        flat = np.concatenate(parts, axis=0).reshape(-1)
        wblocks, pos = [], 0
        for sb in subbatches:
            for (off, k) in sb.calls:
                n = k * P
                blk = flat[pos + off * P: pos + off * P + n]
                wblocks.append(blk.reshape(n // 16, 16).T)
            pos += sb.ntiles * sb.s * P
        w = np.concatenate(wblocks, axis=1)
        gidx_cores.append(np.ascontiguousarray(np.tile(w, (8, 1))))

        # combine idx: item i=(col*128+p) of partial qi's cb block = row in
        # part_d[qi] for the dst at stream-0 position (t=col, p)
        maps = []
        order0 = per_core[c][0]["order"]
        for qi in range(3):
            order_q = per_core[c][qi + 1]["order"]
            posq = np.zeros(own, dtype=np.int64)
            posq[order_q] = np.arange(own)
            j = np.zeros(nloc, dtype=np.int64)
            j[:own] = posq[order0]
            j[own:] = np.arange(own, nloc)    # pads -> stream-q pads (zero)
            rows = (j % P) * nt + (j // P)
            maps.append(rows.astype(np.int16))
        flat = np.concatenate(maps)
        wblocks = []
        for (qi, off, k) in cmb_calls:
            n = k * P
            blk = flat[qi * nloc + off * P: qi * nloc + off * P + n]
            wblocks.append(blk.reshape(n // 16, 16).T)
        w = np.concatenate(wblocks, axis=1)
        cidx_cores.append(np.ascontiguousarray(np.tile(w, (8, 1))))

        dv = np.zeros((P, nt), dtype=np.float32)
        i = np.arange(own)
        dv[i % P, i // P] = dinv[c * own + order0]
        dinv_cores.append(dv)

    # schedules (common): global call list with queue round-robin
    call_rec = []          # (kind, layer, sbi_or_qi, off, k, queue, qseq)
    qcount = [0, 0, 0, 0]
    for l in range(LAYERS):
        for sbi, sb in enumerate(subbatches):
            for (off, k) in sb.calls:
                q = (len(call_rec)) % 4
                call_rec.append(Struct(kind="g", layer=l, sbi=sbi, off=off, k=k,
                                       q=q, qseq=qcount[q]))
                qcount[q] += 1
        for ci, (qi, off, k) in enumerate(cmb_calls):
            q = (len(call_rec)) % 4
            call_rec.append(Struct(kind="c", layer=l, qi=qi, off=off, k=k,
                                   q=q, qseq=qcount[q]))
            qcount[q] += 1

    return Struct(
        n_nodes=n_nodes, ncores=ncores, own=own, nt=nt, nloc=nloc, nloct=nloct,
        ntot=ntot, null_quad=null_quad, subbatches=subbatches,
        cmb_calls=cmb_calls, call_rec=call_rec, total_cols=total_cols,
        gidx=gidx_cores, cidx=cidx_cores, dinv_pm=dinv_cores,
        s0_orders=s0_orders, trow=trow, deg=deg, dinv=dinv,
    )


# ======================= device program =======================

def _raw_dma_gather(gp, bass, mybir, out_ap, in_ap, idxs_ap, num_idxs,
                    elem_size, elem_step, queue_num):
    """dma_gather minus the bass-level elem%256 restriction (ISA only needs a
    256B-granular stride)."""
    from concourse import ap_utils
    assert idxs_ap.dtype == mybir.dt.int16
    assert in_ap.dtype == out_ap.dtype
    dt_size = mybir.dt.size(in_ap.dtype)
    stride_bytes = elem_step * dt_size
    assert stride_bytes % 256 == 0 and stride_bytes // 256 < 256
    assert ap_utils.ap_is_contiguous(in_ap.ap[1:])
    assert ap_utils.ap_is_contiguous(out_ap.ap[1:])
    assert ap_utils.ap_is_contiguous(idxs_ap.ap[1:])
    assert in_ap.ap[0][0] == elem_step
    assert num_idxs % 128 == 0 and num_idxs <= 1024
    assert out_ap.ap[0][1] * out_ap.ap[1][1] == num_idxs
    assert in_ap.ap[-1][1] == out_ap.ap[-1][1] == elem_size
    return gp.add_instruction(
        mybir.InstDMAGatherAnt(
            name=gp.bass.get_next_instruction_name(),
            ins=[*gp.lower_ap_dma(in_ap, for_custom_bir_dma=True),
                 gp.lower_ap(idxs_ap),
                 gp.lower_val_access(gp.to_reg(num_idxs))],
            outs=[gp.lower_ap(out_ap)],
            transpose=False, num_idxs=num_idxs, elem_size=elem_size,
            stride_bytes_256=stride_bytes // 256, gen_mode=0,
            single_packet=True, queue_num=queue_num,
            sbuf_tokens_per_rank=0, sbuf_free_dim_per_rank=0,
            sbuf_free_dim_pad_per_rank=0, sbuf_byte_offset=0,
        ))


def build_nc(st, reps=1, skip_gather=False, skip_reduce=False, skip_ag=False, skip_stage=False, skip_mm=False, shared_gall=True):
    import concourse.bass as bass
    import concourse.bacc as bacc
    import concourse.mybir as mybir
    from concourse import library_config
    import contextlib

    nt, nloc, nloct, ntot = st.nt, st.nloc, st.nloct, st.ntot
    NTQ = ntot // 4
    C16 = st.gidx[0].shape[1]
    CC16 = st.cidx[0].shape[1]
    n_sb = len(st.subbatches)
    sb_stage_cols = [sb.ntiles * sb.s for sb in st.subbatches]
    sb_first_call = []
    acc = 0
    for sb in st.subbatches:
        sb_first_call.append(acc)
        acc += len(sb.calls)
    g_calls_per_layer = acc
    grecs = [r for r in st.call_rec if r.kind == "g"]
    crecs = [r for r in st.call_rec if r.kind == "c"]
    qtot = [0, 0, 0, 0]
    for r in st.call_rec:
        qtot[r.q] += 1

    nc = bacc.Bacc("TRN2", num_swdge_queues=4)
    f16, f32, i16 = mybir.dt.float16, mybir.dt.float32, mybir.dt.int16

    xt_d = nc.declare_dram_parameter("xt", [FIN, nloc], f16, isOutput=False)
    w1_d = nc.declare_dram_parameter("w1", [FIN, F], f16, isOutput=False)
    w2_d = nc.declare_dram_parameter("w2", [F, F], f16, isOutput=False)
    w3b_d = nc.declare_dram_parameter("w3b", [P, F], f32, isOutput=False)
    b1b_d = nc.declare_dram_parameter("b1b", [P, F], f32, isOutput=False)
    b2b_d = nc.declare_dram_parameter("b2b", [P, F], f32, isOutput=False)
    b3b_d = nc.declare_dram_parameter("b3b", [P, 1], f32, isOutput=False)
    dinv_d = nc.declare_dram_parameter("dinv", [P, nt], f32, isOutput=False)
    iden_d = nc.declare_dram_parameter("iden", [P, P], f16, isOutput=False)
    gidx_d = nc.declare_dram_parameter("gidx", [P, C16], i16, isOutput=False)
    cidx_d = nc.declare_dram_parameter("cidx", [P, CC16], i16, isOutput=False)
    z4_d = nc.declare_dram_parameter("z4", [4, F], f16, isOutput=False)
    out_d = nc.declare_dram_parameter("out", [P, nt], f32, isOutput=True)

    gloc_d = nc.dram_tensor("g_loc", [nloct, F], f16)
    if shared_gall:
        gall_d = nc.dram_tensor("g_all", [ntot, F], f16, addr_space="Shared")
    else:
        gall_d = nc.dram_tensor("g_all", [ntot, F], f16)
    part_d = [nc.dram_tensor(f"part{q}", [nloc, 64], f32) for q in (1, 2, 3)]

    with contextlib.ExitStack() as ctx:
        Ecm = ctx.enter_context
        blk = Ecm(nc.Block())
        xt_sb = Ecm(nc.sbuf_tensor("xt_sb", [FIN, nloc], f16))
        act = Ecm(nc.sbuf_tensor("act", [P, nt, F], f32))          # + partial_0
        act16 = Ecm(nc.sbuf_tensor("act16", [P, nt, F], f16))
        g_sb = Ecm(nc.sbuf_tensor("g_sb", [P, nt, F], f16))
        part_sb = [Ecm(nc.sbuf_tensor(f"part_sb{q}", [P, nt, F], f32)) for q in (1, 2, 3)]
        gb = Ecm(nc.sbuf_tensor("gb", [P, 2 * GB_COLS, F], f16))
        rt = Ecm(nc.sbuf_tensor("rt", [P, GB_COLS // 2, F], f32))
        cb = Ecm(nc.sbuf_tensor("cb", [P, 3 * nt, F], f32))
        idxs = Ecm(nc.sbuf_tensor("idxs", [P, 2 * GB_COLS * 8], i16))
        cidxs = Ecm(nc.sbuf_tensor("cidxs", [P, CC16], i16))
        w1_sb = Ecm(nc.sbuf_tensor("w1_sb", [FIN, F], f16))
        w2_sb = Ecm(nc.sbuf_tensor("w2_sb", [F, F], f16))
        w3b_sb = Ecm(nc.sbuf_tensor("w3b_sb", [P, F], f32))
        b1b_sb = Ecm(nc.sbuf_tensor("b1b_sb", [P, F], f32))
        b2b_sb = Ecm(nc.sbuf_tensor("b2b_sb", [P, F], f32))
        b3b_sb = Ecm(nc.sbuf_tensor("b3b_sb", [P, 1], f32))
        dinv_sb = Ecm(nc.sbuf_tensor("dinv_sb", [P, nt], f32))
        ident = Ecm(nc.sbuf_tensor("ident", [P, P], f16))
        actT = [Ecm(nc.sbuf_tensor(f"actT{i}", [F, P], f16)) for i in range(2)]
        outc = Ecm(nc.sbuf_tensor("outc", [P, nt], f32))
        ps_m = [Ecm(nc.psum_tensor(f"ps_m{i}", [P, F], f32)) for i in range(2)]
        ps_t = [Ecm(nc.psum_tensor(f"ps_t{i}", [F, P], f16)) for i in range(2)]

        s_ld = Ecm(nc.semaphore("s_ld"))
        s_mm = Ecm(nc.semaphore("s_mm"))
        s_gv = Ecm(nc.semaphore("s_gv"))
        s_tp = Ecm(nc.semaphore("s_tp"))
        s_tc = Ecm(nc.semaphore("s_tc"))
        s_gsp = Ecm(nc.semaphore("s_gsp"))
        s_ag = Ecm(nc.semaphore("s_ag"))
        s_ix = [Ecm(nc.semaphore(f"s_ix{h}")) for h in range(2)]  # per-half idx staged
        s_cx = Ecm(nc.semaphore("s_cx"))       # combine idx staged (16/layer)
        qsem = [Ecm(nc.semaphore(f"qs{i}")) for i in range(4)]
        s_rd = Ecm(nc.semaphore("s_rd"))       # 1/sub-batch
        s_cmb = Ecm(nc.semaphore("s_cmb"))     # 1/layer: combine adds done
        s_sp = Ecm(nc.semaphore("s_sp"))       # 16/spill
        s_pp = Ecm(nc.semaphore("s_pp"))       # 1/layer postproc done
        s_out = Ecm(nc.semaphore("s_out"))

        N_LOADS = 10

        # -------- sync: loads, staging, spills, output --------
        @blk.sync
        def _(sy):
            for dst_sb, src_d in ((xt_sb, xt_d), (w1_sb, w1_d), (w2_sb, w2_d),
                                  (w3b_sb, w3b_d), (b1b_sb, b1b_d),
                                  (b2b_sb, b2b_d), (b3b_sb, b3b_d),
                                  (dinv_sb, dinv_d), (ident, iden_d)):
                sy.dma_start(out=dst_sb[:, :], in_=src_d[:, :]).then_inc(s_ld, 16)
            sy.dma_start(out=gloc_d[nloc:nloc + 4, :], in_=z4_d[:, :]).then_inc(s_ld, 16)

            def sb_wait_gathers(eng, pi):
                # wait all gather calls of global sub-batch pi
                gl = pi // n_sb          # global layer
                rep_i, l_i = divmod(gl, LAYERS)
                base = l_i * g_calls_per_layer + sb_first_call[pi % n_sb]
                per_q = {}
                for j in range(len(st.subbatches[pi % n_sb].calls)):
                    r = grecs[base + j]
                    per_q[r.q] = rep_i * qtot[r.q] + r.qseq + 1
                for q, cnt in per_q.items():
                    eng.wait_ge(qsem[q], cnt * 16)

            for rep in range(reps):
              for l0 in range(LAYERS):
                l = rep * LAYERS + l0
                sy.wait_ge(s_gv, (l + 1) * nt)
                sy.dma_start(
                    out=gloc_d[0:nloc, :].rearrange("(p t) f -> p (t f)", p=P),
                    in_=g_sb[:, :, :],
                ).then_inc(s_gsp, 16)
                ci = 0
                for sbi, sb in enumerate(st.subbatches):
                    gsbi = l * n_sb + sbi
                    if gsbi >= 2:
                        sb_wait_gathers(sy, gsbi - 2)
                    half = gsbi % 2
                    w = sb_stage_cols[sbi] * 8
                    if skip_stage:
                        sy.sem_inc(s_ix[half], 16)
                    else:
                        sy.dma_start(
                            out=idxs[:, half * GB_COLS * 8: half * GB_COLS * 8 + w],
                            in_=gidx_d[:, ci:ci + w],
                        ).then_inc(s_ix[half], 16)
                    ci += w
                if l >= 1:
                    sy.wait_ge(s_cmb, l)
                sy.dma_start(out=cidxs[:, :], in_=cidx_d[:, :]).then_inc(s_cx, 16)
                sy.wait_ge(s_rd, (l + 1) * n_sb)
                for qi in range(3):
                    sy.dma_start(
                        out=bass.AP(part_d[qi], 0, [[64 * nt, P], [64, nt], [1, F]]),
                        in_=part_sb[qi][:, :, :],
                    ).then_inc(s_sp, 16)
            sy.wait_ge(s_pp, reps * LAYERS)
            sy.dma_start(out=out_d[:, :], in_=outc[:, :]).then_inc(s_out, 16)
            sy.wait_ge(s_out, 16)

        # -------- tensor: transforms --------
        @blk.tensor
        def _(te):
            te.wait_ge(s_ld, 16 * N_LOADS)
            for rep in range(reps):
              for l0 in range(2):
                gvl = rep * LAYERS + l0
                mml = rep * 2 + l0
                if l0 == 1:
                    te.wait_ge(s_pp, rep * LAYERS + 1)
                for t in range(nt):
                    gvt = gvl * nt + t
                    mmt = mml * nt + t
                    if l0 == 0:
                        if gvt >= 2:
                            te.wait_ge(s_gv, gvt - 1)
                        if skip_mm:
                            te.sem_inc(s_mm, 1)
                            continue
                        te.matmul(ps_m[t % 2][:, :],
                                  xt_sb[:, t * P:(t + 1) * P],
                                  w1_sb[:, :]).then_inc(s_mm, 1)
                    else:
                        tpt = rep * nt + t
                        if tpt >= 2:
                            te.wait_ge(s_tc, tpt - 1)
                        te.transpose(ps_t[t % 2][:, :], act16[:, t, :],
                                     ident[:, :]).then_inc(s_tp, 1)
                        te.wait_ge(s_tc, tpt + 1)
                        if gvt >= 2:
                            te.wait_ge(s_gv, gvt - 1)
                        te.matmul(ps_m[t % 2][:, :], actT[t % 2][:, :],
                                  w2_sb[:, :]).then_inc(s_mm, 1)

        # -------- gpsimd: collective + gathers --------
        @blk.gpsimd
        def _(gp):
            gp.load_library(library_config.mlp)
            gp.wait_ge(s_ld, 16 * N_LOADS)
            qcnt = [0, 0, 0, 0]
            for rep in range(reps):
              qbase = [rep * qtot[i] for i in range(4)]
              gi = 0
              ci = 0
              for l0 in range(LAYERS):
                l = rep * LAYERS + l0
                gp.wait_ge(s_gsp, (l + 1) * 16)
                if skip_ag:
                    gp.sem_inc(s_ag, 1)
                else:
                    gp.collective_compute(
                        "AllGather", mybir.AluOpType.bypass,
                        replica_groups=[list(range(st.ncores))],
                        ins=[gloc_d[:, :]],
                        outs=[gall_d[:, :]],
                    ).then_inc(s_ag, 1)
                gp.wait_ge(s_ag, l + 1)
                for sbi, sb in enumerate(st.subbatches):
                    gsbi = l * n_sb + sbi
                    gp.wait_ge(s_ix[gsbi % 2], (gsbi // 2 + 1) * 16)
                    if gsbi >= 2:
                        gp.wait_ge(s_rd, gsbi - 1)
                    half = gsbi % 2
                    col0 = half * GB_COLS
                    ihalf_base = half * GB_COLS * 8
                    for (off, k) in sb.calls:
                        r = grecs[gi]
                        gi += 1
                        if skip_gather:
                            gp.sem_inc(qsem[r.q], 16)
                            qcnt[r.q] = qbase[r.q] + r.qseq + 1
                            continue
                        if qcnt[r.q] >= 1:
                            gp.wait_ge(qsem[r.q], qcnt[r.q] * 16)
                        _raw_dma_gather(
                            gp, bass, mybir,
                            out_ap=gb[:, col0 + off: col0 + off + k, :],
                            in_ap=bass.AP(gall_d, F * sb.q, [[F * 4, NTQ], [1, F]]),
                            idxs_ap=idxs[:, ihalf_base + off * 8: ihalf_base + (off + k) * 8],
                            num_idxs=k * P, elem_size=F, elem_step=F * 4,
                            queue_num=r.q,
                        ).then_inc(qsem[r.q], 16)
                        qcnt[r.q] = qbase[r.q] + r.qseq + 1
                gp.wait_ge(s_sp, (l + 1) * 48)
                gp.wait_ge(s_cx, (l + 1) * 16)
                coff = 0
                for (qi, off, k) in st.cmb_calls:
                    r = crecs[ci]
                    ci += 1
                    if skip_gather:
                        gp.sem_inc(qsem[r.q], 16)
                        qcnt[r.q] = qbase[r.q] + r.qseq + 1
                        coff += k * 8
                        continue
                    if qcnt[r.q] >= 1:
                        gp.wait_ge(qsem[r.q], qcnt[r.q] * 16)
                    _raw_dma_gather(
                        gp, bass, mybir,
                        out_ap=cb[:, qi * nt + off: qi * nt + off + k, :],
                        in_ap=bass.AP(part_d[qi], 0, [[64, nloc], [1, F]]),
                        idxs_ap=cidxs[:, coff: coff + k * 8],
                        num_idxs=k * P, elem_size=F, elem_step=64,
                        queue_num=r.q,
                    ).then_inc(qsem[r.q], 16)
                    qcnt[r.q] = qbase[r.q] + r.qseq + 1
                    coff += k * 8

        # -------- vector: copies, g tiles, reduces, combine, postproc --------
        @blk.vector
        def _(v):
            v.wait_ge(s_ld, 16 * N_LOADS)
            for rep in range(reps):
              qbase = [rep * qtot[i] for i in range(4)]
              gi = 0
              ci = 0
              for l0 in range(LAYERS):
                l = rep * LAYERS + l0
                for t in range(nt):
                    if l0 < 2:
                        mmt = (rep * 2 + l0) * nt + t
                        if l0 == 1:
                            v.wait_ge(s_tp, rep * nt + t + 1)
                            v.tensor_copy(out=actT[t % 2][:, :],
                                          in_=ps_t[t % 2][:, :]).then_inc(s_tc, 1)
                        v.wait_ge(s_mm, mmt + 1)
                        src = ps_m[t % 2][:, :]
                    else:
                        src = act16[:, t, :]
                    v.tensor_tensor(
                        out=g_sb[:, t, :], in0=src,
                        in1=dinv_sb[:, t:t + 1].to_broadcast([P, F]),
                        op=mybir.AluOpType.mult,
                    ).then_inc(s_gv, 1)
                for sbi, sb in enumerate(st.subbatches):
                    gsbi = l * n_sb + sbi
                    half = gsbi % 2
                    col0 = half * GB_COLS
                    per_q = {}
                    for _x in sb.calls:
                        r = grecs[gi]
                        gi += 1
                        per_q[r.q] = qbase[r.q] + r.qseq + 1
                    for q, cnt in per_q.items():
                        v.wait_ge(qsem[q], cnt * 16)
                    if not skip_reduce:
                        _emit_tree(v, bass, mybir, gb, rt,
                                   act if sb.q == 0 else part_sb[sb.q - 1],
                                   col0, sb.t0, sb.ntiles, sb.s, P, F)
                    v.sem_inc(s_rd, 1)
                per_q = {}
                for _x in st.cmb_calls:
                    r = crecs[ci]
                    ci += 1
                    per_q[r.q] = qbase[r.q] + r.qseq + 1
                for q, cnt in per_q.items():
                    v.wait_ge(qsem[q], cnt * 16)
                for qi in range(3):
                    v.tensor_tensor(out=act[:, :, :], in0=act[:, :, :],
                                    in1=cb[:, qi * nt:(qi + 1) * nt, :],
                                    op=mybir.AluOpType.add)
                    v.drain()
                v.sem_inc(s_cmb, 1)
                v.tensor_tensor(
                    out=act[:, :, :], in0=act[:, :, :],
                    in1=bass.AP(dinv_sb, 0,
                                [[dinv_sb.ap().ap[0][0], P], [1, nt], [0, F]]),
                    op=mybir.AluOpType.mult)
                v.drain()
                if l0 < 2:
                    bias = b1b_sb if l0 == 0 else b2b_sb
                    v.tensor_tensor(
                        out=act[:, :, :], in0=act[:, :, :],
                        in1=bass.AP(bias, 0,
                                    [[bias.ap().ap[0][0], P], [0, nt], [1, F]]),
                        op=mybir.AluOpType.add)
                    v.drain()
                    v.tensor_scalar_max(out=act[:, :, :], in0=act[:, :, :],
                                        scalar1=0.0)
                    v.drain()
                    v.tensor_copy(out=act16[:, :, :], in_=act[:, :, :]).then_inc(s_pp, 1)
                    v.drain()
                else:
                    v.tensor_tensor(
                        out=rt[:, 0:nt, :], in0=act[:, :, :],
                        in1=bass.AP(w3b_sb, 0,
                                    [[w3b_sb.ap().ap[0][0], P], [0, nt], [1, F]]),
                        op=mybir.AluOpType.mult)
                    v.drain()
                    v.tensor_reduce(out=outc[:, :], in_=rt[:, 0:nt, :],
                                    axis=mybir.AxisListType.X,
                                    op=mybir.AluOpType.add)
                    v.drain()
                    v.tensor_tensor(
                        out=outc[:, :], in0=outc[:, :],
                        in1=bass.AP(b3b_sb, 0,
                                    [[b3b_sb.ap().ap[0][0], P], [0, nt]]),
                        op=mybir.AluOpType.add).then_inc(s_pp, 1)

    nc.compile()
    return nc


def _emit_tree(v, bass, mybir, gb, rt, target, col0, t0, ntl, s, P, F):
    """Sum s slot-columns per tile (f16 in gb) into target[:, t0:t0+ntl, :] f32."""
    _eng = v
    def tensor_tensor_drained(**kw):
        r = _eng.tensor_tensor(**kw)
        _eng.drain()
        return r
    class _V:
        tensor_tensor = staticmethod(tensor_tensor_drained)
    v = _V()
    gp0 = gb.ap().ap[0][0]
    rp0 = rt.ap().ap[0][0]
    tp0 = target.ap().ap[0][0]
    add = mybir.AluOpType.add

    def gbap(coff, tilestride, cols):
        return bass.AP(gb, (col0 + coff) * F,
                       [[gp0, P], [tilestride * F, ntl], [F, cols], [1, F]])

    def rtap(coff, tilestride, cols):
        return bass.AP(rt, coff * F,
                       [[rp0, P], [tilestride * F, ntl], [F, cols], [1, F]])

    def tgap():
        return bass.AP(target, t0 * F, [[tp0, P], [F, ntl], [1, F]])

    if s == 2:
        v.tensor_tensor(out=tgap(), in0=gbap(0, s, 1), in1=gbap(1, s, 1), op=add)
        return
    h = s // 2
    odd = s % 2
    rstride = h
    # level 0: gb -> rt
    if h == 1:
        v.tensor_tensor(out=tgap(), in0=gbap(0, s, 1), in1=gbap(1, s, 1), op=add)
        if odd:
            v.tensor_tensor(out=tgap(), in0=tgap(), in1=gbap(2, s, 1), op=add)
        return
    v.tensor_tensor(out=rtap(0, rstride, h), in0=gbap(0, s, h),
                    in1=gbap(h, s, h), op=add)
    if odd:
        v.tensor_tensor(out=rtap(0, rstride, 1), in0=rtap(0, rstride, 1),
                        in1=gbap(2 * h, s, 1), op=add)
    k = h
    while k > 1:
        h2 = k // 2
        odd2 = k % 2
        if h2 == 1:
            v.tensor_tensor(out=tgap(), in0=rtap(0, rstride, 1),
                            in1=rtap(1, rstride, 1), op=add)
            if odd2:
                v.tensor_tensor(out=tgap(), in0=tgap(),
                                in1=rtap(2, rstride, 1), op=add)
            return
        v.tensor_tensor(out=rtap(0, rstride, h2), in0=rtap(0, rstride, h2),
                        in1=rtap(h2, rstride, h2), op=add)
        if odd2:
            v.tensor_tensor(out=rtap(0, rstride, 1), in0=rtap(0, rstride, 1),
                            in1=rtap(2 * h2, rstride, 1), op=add)
        k = h2


# ======================= top-level kernel =======================

def _build_inputs(st, x, W1, b1, W2, b2, W3, b3):
    in_maps = []
    eye = np.eye(P, dtype=np.float16)
    z4 = np.zeros((4, F), dtype=np.float16)
    for c in range(st.ncores):
        order0 = st.s0_orders[c]
        xs = np.zeros((st.nloc, FIN), dtype=np.float16)
        # row r = p*nt + t holds x of dst at stream-0 position t*128+p;
        # xt columns are TILE-major (t*128+p) for contiguous matmul lhsT.
        xloc = x[c * st.own + order0].astype(np.float16)    # [own, FIN] in s0 order
        xt = np.zeros((FIN, st.nloc), dtype=np.float16)
        xt[:, :st.own] = xloc.T                              # col j = s0 position j
        in_maps.append(dict(
            xt=np.ascontiguousarray(xt),
            w1=W1.astype(np.float16),
            w2=W2.astype(np.float16),
            w3b=np.ascontiguousarray(np.tile(W3[:, 0][None, :], (P, 1)).astype(np.float32)),
            b1b=np.ascontiguousarray(np.tile(b1[None, :], (P, 1)).astype(np.float32)),
            b2b=np.ascontiguousarray(np.tile(b2[None, :], (P, 1)).astype(np.float32)),
            b3b=np.full((P, 1), float(b3[0]), dtype=np.float32),
            dinv=st.dinv_pm[c],
            iden=eye,
            gidx=st.gidx[c],
            cidx=st.cidx[c],
            z4=z4,
        ))
    return in_maps


def kernel(x, edge_index, W1, b1, W2, b2, W3, b3):
    from concourse.bass_utils import run_bass_kernel_spmd
    x = np.asarray(x)
    st = prep(np.asarray(edge_index))
    nc = build_nc(st)
    in_maps = _build_inputs(st, x, np.asarray(W1), np.asarray(b1),
                            np.asarray(W2), np.asarray(b2),
                            np.asarray(W3), np.asarray(b3))
    res = run_bass_kernel_spmd(nc, in_maps, list(range(st.ncores)))
    out = np.zeros(st.n_nodes, dtype=np.float32)
    for c in range(st.ncores):
        oc = res.results[c]["out"]          # [P, nt]
        order0 = st.s0_orders[c]
        i = np.arange(st.own)
        out[c * st.own + order0] = oc[i % P, i // P]
    return out



# revision 38
# speedup vs baseline: 59.4388x; 59.4388x over previous
"""GCN (3-layer DemandGNN) Trainium2 Bass kernel — 8-core SPMD.

Node-sharded GCN: each core owns 12500 destination nodes. Per layer:
 - transform activations locally on PE (x@W1 / act@W2), pre-scale by
   dinv (symmetric norm, source side), cast fp16;
 - AllGather the fp16 feature table into each core's DRAM;
 - gather per-edge source rows with batched SWDGE dma_gather: 64-byte
   descriptors over a 256B-strided quad view of the table, 4 residue
   streams (table row % 4) selected by call byte-offset;
 - DVE tree-reduce padded per-dst slot groups into 4 per-stream partials;
 - align partials (each stream has its own dst order) with a small
   combine gather, add self-loop term (act += g_sb), then dinv (dst
   side) + bias + relu;
 - layer 3 aggregates act2*dinv and applies W3 afterwards (A(hW)=(Ah)W).

Gather idx data is RESIDENT in SBUF (loaded once): each call's wrapped
[16, 8k] idx block lives in the partition band [32q, 32q+32) of its
fixed SWDGE queue q (the ucode's Q7 core pair 2q/2q+1 reads only that
band), replicated x2 within the band. Queue assignment is
layer-independent, so one copy serves all layers and reps. Per-queue
gather pipelining uses `queue_depth` rotating completion sems (a single
sem's count can be satisfied by a later call's unordered per-engine
increments, which corrupts at depth>=2 — rotating sems fix that).
Slot counts are exact per tile (max need over cores, min 2), self-loops
are excluded from the edge streams and applied locally.

All structure (tile slot classes, call layout) is shared across cores so
one SPMD program serves all 8; only index/feature data differ per core.
"""
import numpy as np

P = 128
F = 32            # hidden feature dim
FIN = 128         # input feature dim
NCORES = 8
N_NODES = 100000

CALL_COLS = 8     # slot columns per dma_gather call (idxs = 128*cols <= ring)
GB_COLS = 240     # gather buffer columns per half (double buffered)
IDX_HALVES = 2    # idx staging buffers (prefetch depth)
QUEUE_DEPTH = 3   # outstanding gather calls per SWDGE queue (rotating sems)
LADDER = [2, 4, 6, 8, 10, 12, 14, 16, 18, 20, 22, 24, 26, 28, 30, 32,
          36, 40, 44, 48, 56, 64, 80, 96, 128, 160, 192, 240]
LAYERS = 3


def _ladder_round(x):
    for v in LADDER:
        if x <= v:
            return v
    raise ValueError(f"slot count {x} exceeds ladder")


class Struct(dict):
    __getattr__ = dict.__getitem__
    __setattr__ = dict.__setitem__


# ======================= host preprocessing =======================

def prep(edge_index, n_nodes=N_NODES, ncores=NCORES, call_cols=CALL_COLS,
         nqueues=4):
    own = n_nodes // ncores
    nt = (own + P - 1) // P
    nloc = nt * P
    nloct = nloc + 4                  # +4 null rows (cover all residues)
    ntot = ncores * nloct
    null_quad = nloc // 4             # core0 rows nloc..nloc+3 (zeros)

    # streams hold real edges only; self-loops are added on-device
    # (act += g_sb before the dst-side dinv), but deg includes them.
    src = np.asarray(edge_index[0], dtype=np.int64)
    dst = np.asarray(edge_index[1], dtype=np.int64)
    deg = np.bincount(dst, minlength=n_nodes).astype(np.int64) + 1
    dinv = (1.0 / np.sqrt(deg.astype(np.float64))).astype(np.float32)

    core_of_edge = dst // own

    # stream-0 dst order: by total degree (residue-independent to avoid
    # circularity — table rows depend on it, residues depend on table rows)
    s0_orders = []
    s0pos_row = np.zeros(n_nodes, dtype=np.int64)   # p-major row within core
    for c in range(ncores):
        cnt = np.bincount(dst[core_of_edge == c] - c * own, minlength=own)
        order0 = np.argsort(-cnt, kind="stable")
        s0_orders.append(order0)
        pos = np.zeros(own, dtype=np.int64)
        pos[order0] = np.arange(own)
        s0pos_row[c * own:(c + 1) * own] = (pos % P) * nt + (pos // P)

    trow = (np.arange(n_nodes) // own) * nloct + s0pos_row
    vres = trow % 4
    vquad = trow // 4
    assert vquad.max() < 32768

    per_core = []
    need = np.zeros((ncores, 4, nt), dtype=np.int64)
    for c in range(ncores):
        m = core_of_edge == c
        e_src, e_dst_l = src[m], dst[m] - c * own
        streams = []
        for q in range(4):
            mq = vres[e_src] == q
            cq = np.bincount(e_dst_l[mq], minlength=own)
            if q == 0:
                order_q = s0_orders[c]
            else:
                order_q = np.argsort(-cq, kind="stable")
            cq_s = np.concatenate([cq[order_q], np.zeros(nloc - own, np.int64)])
            need[c, q] = cq_s.reshape(nt, P).max(axis=1)
            oq = np.argsort(e_dst_l[mq], kind="stable")
            streams.append(dict(order=order_q,
                                eq_quad=vquad[e_src[mq]][oq],
                                st=np.concatenate([[0], np.cumsum(cq)]),
                                cnt=cq))
        per_core.append(streams)

    tiles_s = np.zeros((4, nt), dtype=np.int64)
    for q in range(4):
        for t in range(nt):
            tiles_s[q, t] = max(2, int(need[:, q, t].max()))

    # sub-batches: runs of equal s (per stream), tile-aligned, <= GB_COLS
    subbatches = []
    for q in range(4):
        t = 0
        while t < nt:
            s = int(tiles_s[q, t])
            assert s <= GB_COLS
            t1 = t
            while t1 < nt and int(tiles_s[q, t1]) == s and (t1 - t + 1) * s <= GB_COLS:
                t1 += 1
            calls = []
            cols = (t1 - t) * s
            off = 0
            while off < cols:
                k = min(call_cols, cols - off)
                calls.append((off, k))
                off += k
            subbatches.append(Struct(q=q, t0=t, ntiles=t1 - t, s=s, calls=calls))
            t = t1
    total_cols = sum(sb.ntiles * sb.s for sb in subbatches)

    # combine calls (common): 3 partials x nt columns
    cmb_calls = []
    for qi in range(3):
        off = 0
        while off < nt:
            k = min(call_cols, nt - off)
            cmb_calls.append((qi, off, k))
            off += k

    # schedules (common): queue round-robin, layer-INDEPENDENT so each call's
    # idx block lives in one fixed queue band of the resident idx tensor.
    # Q7 cores 2q/2q+1 serve queue q and read idxs from partitions
    # [32q, 32q+32) only, so a call's idx block is replicated x2 there.
    call_rec = []          # (kind, layer, sbi_or_qi, off, k, queue, qseq, ioff)
    qcount = [0, 0, 0, 0]
    band_pos = [0, 0, 0, 0]   # gather-idx band fill (cols)
    cband_pos = [0, 0, 0, 0]  # combine-idx band fill (cols)
    for l in range(LAYERS):
        ci_l = 0
        for sbi, sb in enumerate(subbatches):
            for (off, k) in sb.calls:
                q = ci_l % nqueues
                ci_l += 1
                if l == 0:
                    ioff = band_pos[q]
                    band_pos[q] += 8 * k
                else:
                    ioff = call_rec[len(call_rec) - n_calls_l].ioff
                call_rec.append(Struct(kind="g", layer=l, sbi=sbi, off=off, k=k,
                                       q=q, qseq=qcount[q], ioff=ioff))
                qcount[q] += 1
        for (qi, off, k) in cmb_calls:
            q = ci_l % nqueues
            ci_l += 1
            if l == 0:
                ioff = cband_pos[q]
                cband_pos[q] += 8 * k
            else:
                ioff = call_rec[len(call_rec) - n_calls_l].ioff
            call_rec.append(Struct(kind="c", layer=l, qi=qi, off=off, k=k,
                                   q=q, qseq=qcount[q], ioff=ioff))
            qcount[q] += 1
        if l == 0:
            n_calls_l = ci_l
    gW = max(band_pos)
    cW = max(cband_pos)

    # ---- per-core gather idx, packed into per-queue bands ----
    gidx_cores, cidx_cores, dinv_cores = [], [], []
    l0_recs = [r for r in call_rec if r.layer == 0]
    for c in range(ncores):
        parts = []
        for sb in subbatches:
            stq = per_core[c][sb.q]
            order_q, s = stq["order"], sb.s
            tc = np.full((sb.ntiles * s, P), null_quad, dtype=np.int16)
            for ti in range(sb.ntiles):
                for p in range(P):
                    gi = (sb.t0 + ti) * P + p
                    if gi >= own:
                        continue
                    d = order_q[gi]
                    k = int(stq["cnt"][d])
                    if k:
                        tc[ti * s:ti * s + k, p] = stq["eq_quad"][stq["st"][d]:stq["st"][d] + k]
            parts.append(tc)
        flat = np.concatenate(parts, axis=0).reshape(-1)

        gidx = np.zeros((P, gW), dtype=np.int16)
        pos = 0
        sbi_prev = -1
        gi = 0
        for r in l0_recs:
            if r.kind != "g":
                continue
            if r.sbi != sbi_prev:
                if sbi_prev >= 0:
                    sb_ = subbatches[sbi_prev]
                    pos += sb_.ntiles * sb_.s * P
                sbi_prev = r.sbi
            n = r.k * P
            blk = flat[pos + r.off * P: pos + r.off * P + n].reshape(n // 16, 16).T
            for rep2 in range(2):
                gidx[32 * r.q + 16 * rep2: 32 * r.q + 16 * rep2 + 16,
                     r.ioff: r.ioff + 8 * r.k] = blk
        gidx_cores.append(np.ascontiguousarray(gidx))

        # combine idx: item i=(col*128+p) of partial qi's cb block = row in
        # part_d[qi] for the dst at stream-0 position (t=col, p)
        maps = []
        order0 = per_core[c][0]["order"]
        for qi in range(3):
            order_q = per_core[c][qi + 1]["order"]
            posq = np.zeros(own, dtype=np.int64)
            posq[order_q] = np.arange(own)
            j = np.zeros(nloc, dtype=np.int64)
            j[:own] = posq[order0]
            j[own:] = np.arange(own, nloc)    # pads -> stream-q pads (zero)
            rows = (j % P) * nt + (j // P)
            maps.append(rows.astype(np.int16))
        cflat = np.concatenate(maps)
        cidx = np.zeros((P, cW), dtype=np.int16)
        for r in l0_recs:
            if r.kind != "c":
                continue
            n = r.k * P
            blk = cflat[r.qi * nloc + r.off * P: r.qi * nloc + r.off * P + n]
            blk = blk.reshape(n // 16, 16).T
            for rep2 in range(2):
                cidx[32 * r.q + 16 * rep2: 32 * r.q + 16 * rep2 + 16,
                     r.ioff: r.ioff + 8 * r.k] = blk
        cidx_cores.append(np.ascontiguousarray(cidx))

        dv = np.zeros((P, nt), dtype=np.float32)
        i = np.arange(own)
        dv[i % P, i // P] = dinv[c * own + order0]
        dinv_cores.append(dv)

    return Struct(
        n_nodes=n_nodes, ncores=ncores, own=own, nt=nt, nloc=nloc, nloct=nloct,
        ntot=ntot, null_quad=null_quad, subbatches=subbatches,
        cmb_calls=cmb_calls, call_rec=call_rec, total_cols=total_cols,
        gW=gW, cW=cW,
        gidx=gidx_cores, cidx=cidx_cores, dinv_pm=dinv_cores,
        s0_orders=s0_orders, trow=trow, deg=deg, dinv=dinv,
    )


# ======================= device program =======================

def _raw_dma_gather(gp, bass, mybir, out_ap, in_ap, idxs_ap, num_idxs,
                    elem_size, elem_step, queue_num, single_packet=True,
                    probe=False):
    """dma_gather minus the bass-level elem%256 restriction (ISA only needs a
    256B-granular stride)."""
    from concourse import ap_utils
    assert idxs_ap.dtype == mybir.dt.int16
    assert in_ap.dtype == out_ap.dtype
    dt_size = mybir.dt.size(in_ap.dtype)
    stride_bytes = elem_step * dt_size
    assert stride_bytes % 256 == 0 and stride_bytes // 256 < 256
    assert ap_utils.ap_is_contiguous(in_ap.ap[1:])
    assert ap_utils.ap_is_contiguous(out_ap.ap[1:])
    assert ap_utils.ap_is_contiguous(idxs_ap.ap[1:])
    assert in_ap.ap[0][0] == elem_step
    assert num_idxs % 128 == 0 and num_idxs <= 1024  # >1024 hangs the ucode
    assert out_ap.ap[0][1] * out_ap.ap[1][1] == num_idxs or probe
    assert (in_ap.ap[-1][1] == out_ap.ap[-1][1] == elem_size) or probe
    return gp.add_instruction(
        mybir.InstDMAGatherAnt(
            name=gp.bass.get_next_instruction_name(),
            ins=[*gp.lower_ap_dma(in_ap, for_custom_bir_dma=True),
                 gp.lower_ap(idxs_ap),
                 gp.lower_val_access(gp.to_reg(num_idxs))],
            outs=[gp.lower_ap(out_ap)],
            transpose=False, num_idxs=num_idxs, elem_size=elem_size,
            stride_bytes_256=stride_bytes // 256, gen_mode=0,
            single_packet=single_packet, queue_num=queue_num,
            sbuf_tokens_per_rank=0, sbuf_free_dim_per_rank=0,
            sbuf_free_dim_pad_per_rank=0, sbuf_byte_offset=0,
        ))


def build_nc(st, reps=1, skip_gather=False, skip_reduce=False, skip_ag=False,
             skip_stage=False, skip_mm=False, shared_gall=True,
             idx_halves=IDX_HALVES, queue_depth=QUEUE_DEPTH,
             elem_probe=None, single_packet=True):
    import concourse.bass as bass
    import concourse.bacc as bacc
    import concourse.mybir as mybir
    from concourse import library_config
    import contextlib

    nt, nloc, nloct, ntot = st.nt, st.nloc, st.nloct, st.ntot
    NTQ = ntot // 4
    C16 = st.gidx[0].shape[1]
    CC16 = st.cidx[0].shape[1]
    n_sb = len(st.subbatches)
    # s_rd count at which stream q's reduces are all done (within a layer)
    stream_end = [0] * 4
    for i, sb in enumerate(st.subbatches):
        stream_end[sb.q] = i + 1
    sb_first_call = []
    acc = 0
    for sb in st.subbatches:
        sb_first_call.append(acc)
        acc += len(sb.calls)
    g_calls_per_layer = acc
    grecs = [r for r in st.call_rec if r.kind == "g"]
    crecs = [r for r in st.call_rec if r.kind == "c"]
    qtot = [0, 0, 0, 0]
    for r in st.call_rec:
        qtot[r.q] += 1


    nc = bacc.Bacc("TRN2", num_swdge_queues=4)
    f16, f32, i16 = mybir.dt.float16, mybir.dt.float32, mybir.dt.int16

    xt_d = nc.declare_dram_parameter("xt", [FIN, nloc], f16, isOutput=False)
    w1_d = nc.declare_dram_parameter("w1", [FIN, F], f16, isOutput=False)
    w2_d = nc.declare_dram_parameter("w2", [F, F], f16, isOutput=False)
    w3b_d = nc.declare_dram_parameter("w3b", [P, F], f32, isOutput=False)
    b1b_d = nc.declare_dram_parameter("b1b", [P, F], f32, isOutput=False)
    b2b_d = nc.declare_dram_parameter("b2b", [P, F], f32, isOutput=False)
    b3b_d = nc.declare_dram_parameter("b3b", [P, 1], f32, isOutput=False)
    dinv_d = nc.declare_dram_parameter("dinv", [P, nt], f32, isOutput=False)
    iden_d = nc.declare_dram_parameter("iden", [P, P], f16, isOutput=False)
    gidx_d = nc.declare_dram_parameter("gidx", [P, C16], i16, isOutput=False)
    cidx_d = nc.declare_dram_parameter("cidx", [P, CC16], i16, isOutput=False)
    z4_d = nc.declare_dram_parameter("z4", [4, F], f16, isOutput=False)
    out_d = nc.declare_dram_parameter("out", [P, nt], f32, isOutput=True)

    gloc_d = nc.dram_tensor("g_loc", [nloct, F], f16)
    if shared_gall:
        gall_d = nc.dram_tensor("g_all", [ntot, F], f16, addr_space="Shared")
    else:
        gall_d = nc.dram_tensor("g_all", [ntot, F], f16)
    part_d = [nc.dram_tensor(f"part{q}", [nloc, 64], f32) for q in (1, 2, 3)]

    with contextlib.ExitStack() as ctx:
        Ecm = ctx.enter_context
        blk = Ecm(nc.Block())
        xt_sb = Ecm(nc.sbuf_tensor("xt_sb", [FIN, nloc], f16))
        act = Ecm(nc.sbuf_tensor("act", [P, nt, F], f32))          # + partial_0
        act16 = Ecm(nc.sbuf_tensor("act16", [P, nt, F], f16))
        g_sb = Ecm(nc.sbuf_tensor("g_sb", [P, nt, F], f16))
        part_sb = [Ecm(nc.sbuf_tensor(f"part_sb{q}", [P, nt, F], f32)) for q in (1, 2, 3)]
        gb = Ecm(nc.sbuf_tensor("gb", [P, 2 * GB_COLS, F], f16))
        rt = Ecm(nc.sbuf_tensor("rt", [P, GB_COLS // 2, F], f32))
        cb = Ecm(nc.sbuf_tensor("cb", [P, 3 * nt, F], f32))
        idxs = Ecm(nc.sbuf_tensor("idxs", [P, C16], i16))       # resident, banded
        cidxs = Ecm(nc.sbuf_tensor("cidxs", [P, CC16], i16))    # resident, banded
        w1_sb = Ecm(nc.sbuf_tensor("w1_sb", [FIN, F], f16))
        w2_sb = Ecm(nc.sbuf_tensor("w2_sb", [F, F], f16))
        w3b_sb = Ecm(nc.sbuf_tensor("w3b_sb", [P, F], f32))
        b1b_sb = Ecm(nc.sbuf_tensor("b1b_sb", [P, F], f32))
        b2b_sb = Ecm(nc.sbuf_tensor("b2b_sb", [P, F], f32))
        b3b_sb = Ecm(nc.sbuf_tensor("b3b_sb", [P, 1], f32))
        dinv_sb = Ecm(nc.sbuf_tensor("dinv_sb", [P, nt], f32))
        ident = Ecm(nc.sbuf_tensor("ident", [P, P], f16))
        actT = [Ecm(nc.sbuf_tensor(f"actT{i}", [F, P], f16)) for i in range(2)]
        outc = Ecm(nc.sbuf_tensor("outc", [P, nt], f32))
        ps_m = [Ecm(nc.psum_tensor(f"ps_m{i}", [P, F], f32)) for i in range(2)]
        ps_t = [Ecm(nc.psum_tensor(f"ps_t{i}", [F, P], f16)) for i in range(2)]

        s_ld = Ecm(nc.semaphore("s_ld"))
        s_mm = Ecm(nc.semaphore("s_mm"))
        s_gv = Ecm(nc.semaphore("s_gv"))
        s_tp = Ecm(nc.semaphore("s_tp"))
        s_tc = Ecm(nc.semaphore("s_tc"))
        s_gsp = Ecm(nc.semaphore("s_gsp"))
        s_ag = Ecm(nc.semaphore("s_ag"))
        # queue_depth sems per queue, rotating by call index: a single qsem's
        # count can be satisfied by a LATER call's per-engine increments
        # (16 incs/call arrive unordered across calls), so depth-D needs
        # D rotating sems to know call N specifically has fully landed.
        nsem_q = max(2, queue_depth)
        qsem = [[Ecm(nc.semaphore(f"qs{i}_{p}")) for p in range(nsem_q)]
                for i in range(4)]
        s_rd = Ecm(nc.semaphore("s_rd"))       # 1/sub-batch
        s_sp = Ecm(nc.semaphore("s_sp"))       # 16/spill
        s_pp = Ecm(nc.semaphore("s_pp"))       # 1/layer postproc done
        s_out = Ecm(nc.semaphore("s_out"))

        N_LOADS = 12
        nsp = max(2, queue_depth)
        qtot_par = [[0] * nsp for _ in range(4)]
        for r in st.call_rec:
            qtot_par[r.q][r.qseq % nsp] += 1

        # -------- sync: loads, spills, output --------
        @blk.sync
        def _(sy):
            for dst_sb, src_d in ((xt_sb, xt_d), (w1_sb, w1_d), (w2_sb, w2_d),
                                  (w3b_sb, w3b_d), (b1b_sb, b1b_d),
                                  (b2b_sb, b2b_d), (b3b_sb, b3b_d),
                                  (dinv_sb, dinv_d), (ident, iden_d),
                                  (cidxs, cidx_d), (idxs, gidx_d)):
                sy.dma_start(out=dst_sb[:, :], in_=src_d[:, :]).then_inc(s_ld, 16)
            sy.dma_start(out=gloc_d[nloc:nloc + 4, :], in_=z4_d[:, :]).then_inc(s_ld, 16)

            for rep in range(reps):
              for l0 in range(LAYERS):
                l = rep * LAYERS + l0
                sy.wait_ge(s_gv, (l + 1) * nt)
                sy.dma_start(
                    out=gloc_d[0:nloc, :].rearrange("(p t) f -> p (t f)", p=P),
                    in_=g_sb[:, :, :],
                ).then_inc(s_gsp, 16)
                for qi in range(3):
                    sy.wait_ge(s_rd, l * n_sb + stream_end[qi + 1])
                    sy.dma_start(
                        out=bass.AP(part_d[qi], 0, [[64 * nt, P], [64, nt], [1, F]]),
                        in_=part_sb[qi][:, :, :],
                    ).then_inc(s_sp, 16)
            sy.wait_ge(s_pp, reps * LAYERS)
            sy.dma_start(out=out_d[:, :], in_=outc[:, :]).then_inc(s_out, 16)
            sy.wait_ge(s_out, 16)

        # -------- tensor: transforms --------
        @blk.tensor
        def _(te):
            te.wait_ge(s_ld, 16 * N_LOADS)
            for rep in range(reps):
              for l0 in range(2):
                gvl = rep * LAYERS + l0
                mml = rep * 2 + l0
                if l0 == 1:
                    te.wait_ge(s_pp, rep * LAYERS + 1)
                for t in range(nt):
                    gvt = gvl * nt + t
                    mmt = mml * nt + t
                    if l0 == 0:
                        if gvt >= 2:
                            te.wait_ge(s_gv, gvt - 1)
                        if skip_mm:
                            te.sem_inc(s_mm, 1)
                            continue
                        te.matmul(ps_m[t % 2][:, :],
                                  xt_sb[:, t * P:(t + 1) * P],
                                  w1_sb[:, :]).then_inc(s_mm, 1)
                    else:
                        tpt = rep * nt + t
                        if tpt >= 2:
                            te.wait_ge(s_tc, tpt - 1)
                        te.transpose(ps_t[t % 2][:, :], act16[:, t, :],
                                     ident[:, :]).then_inc(s_tp, 1)
                        te.wait_ge(s_tc, tpt + 1)
                        if gvt >= 2:
                            te.wait_ge(s_gv, gvt - 1)
                        te.matmul(ps_m[t % 2][:, :], actT[t % 2][:, :],
                                  w2_sb[:, :]).then_inc(s_mm, 1)

        # -------- gpsimd: collective + gathers --------
        @blk.gpsimd
        def _(gp):
            gp.load_library(library_config.mlp)
            gp.wait_ge(s_ld, 16 * N_LOADS)
            qlast = [None] * 4           # (parity, global parity idx) last issued

            def gather_wait(r, rep):
                p = r.qseq % nsp
                gpar = rep * qtot_par[r.q][p] + r.qseq // nsp
                if queue_depth == 1:
                    if qlast[r.q] is not None:
                        lp, lidx = qlast[r.q]
                        gp.wait_ge(qsem[r.q][lp], (lidx + 1) * 16)
                elif gpar >= 1:
                    gp.wait_ge(qsem[r.q][p], gpar * 16)
                qlast[r.q] = (p, gpar)
                return p

            for rep in range(reps):
              gi = 0
              ci = 0
              for l0 in range(LAYERS):
                l = rep * LAYERS + l0
                gp.wait_ge(s_gsp, (l + 1) * 16)
                if skip_ag:
                    gp.sem_inc(s_ag, 1)
                else:
                    gp.collective_compute(
                        "AllGather", mybir.AluOpType.bypass,
                        replica_groups=[list(range(st.ncores))],
                        ins=[gloc_d[:, :]],
                        outs=[gall_d[:, :]],
                    ).then_inc(s_ag, 1)
                gp.wait_ge(s_ag, l + 1)
                for sbi, sb in enumerate(st.subbatches):
                    gsbi = l * n_sb + sbi
                    if gsbi >= 2:
                        gp.wait_ge(s_rd, gsbi - 1)
                    half = gsbi % 2
                    col0 = half * GB_COLS
                    for (off, k) in sb.calls:
                        r = grecs[gi]
                        gi += 1
                        if skip_gather:
                            gp.sem_inc(qsem[r.q][r.qseq % nsp], 16)
                            continue
                        p = gather_wait(r, rep)
                        ep = elem_probe or F
                        _raw_dma_gather(
                            gp, bass, mybir,
                            out_ap=gb[:, col0 + off: col0 + off + k, :],
                            in_ap=bass.AP(gall_d, F * sb.q, [[F * 4, NTQ], [1, ep]]),
                            idxs_ap=idxs[:, r.ioff: r.ioff + 8 * k],
                            num_idxs=k * P, elem_size=ep, elem_step=F * 4,
                            queue_num=r.q, single_packet=single_packet,
                            probe=elem_probe is not None,
                        ).then_inc(qsem[r.q][p], 16)
                for (qi, off, k) in st.cmb_calls:
                    gp.wait_ge(s_sp, (l * 3 + qi + 1) * 16)
                    r = crecs[ci]
                    ci += 1
                    if skip_gather:
                        gp.sem_inc(qsem[r.q][r.qseq % nsp], 16)
                        continue
                    p = gather_wait(r, rep)
                    _raw_dma_gather(
                        gp, bass, mybir,
                        out_ap=cb[:, qi * nt + off: qi * nt + off + k, :],
                        in_ap=bass.AP(part_d[qi], 0, [[64, nloc], [1, F]]),
                        idxs_ap=cidxs[:, r.ioff: r.ioff + 8 * k],
                        num_idxs=k * P, elem_size=F, elem_step=64,
                        queue_num=r.q,
                    ).then_inc(qsem[r.q][p], 16)

        # -------- vector: copies, g tiles, reduces, combine, postproc --------
        @blk.vector
        def _(v):
            v.wait_ge(s_ld, 16 * N_LOADS)

            def wait_calls(recs):
                per_qp = {}
                for r in recs:
                    p = r.qseq % nsp
                    per_qp[(r.q, p)] = rep * qtot_par[r.q][p] + r.qseq // nsp + 1
                for (q, p), cnt in per_qp.items():
                    v.wait_ge(qsem[q][p], cnt * 16)

            for rep in range(reps):
              gi = 0
              ci = 0
              for l0 in range(LAYERS):
                l = rep * LAYERS + l0
                for t in range(nt):
                    if l0 < 2:
                        mmt = (rep * 2 + l0) * nt + t
                        if l0 == 1:
                            v.wait_ge(s_tp, rep * nt + t + 1)
                            v.tensor_copy(out=actT[t % 2][:, :],
                                          in_=ps_t[t % 2][:, :]).then_inc(s_tc, 1)
                        v.wait_ge(s_mm, mmt + 1)
                        src = ps_m[t % 2][:, :]
                    else:
                        src = act16[:, t, :]
                    v.tensor_tensor(
                        out=g_sb[:, t, :], in0=src,
                        in1=dinv_sb[:, t:t + 1].to_broadcast([P, F]),
                        op=mybir.AluOpType.mult,
                    ).then_inc(s_gv, 1)
                for sbi, sb in enumerate(st.subbatches):
                    gsbi = l * n_sb + sbi
                    half = gsbi % 2
                    col0 = half * GB_COLS
                    wait_calls(grecs[gi:gi + len(sb.calls)])
                    gi += len(sb.calls)
                    if not skip_reduce:
                        _emit_tree(v, bass, mybir, gb, rt,
                                   act if sb.q == 0 else part_sb[sb.q - 1],
                                   col0, sb.t0, sb.ntiles, sb.s, P, F)
                    v.sem_inc(s_rd, 1)
                wait_calls(crecs[ci:ci + len(st.cmb_calls)])
                ci += len(st.cmb_calls)
                for qi in range(3):
                    v.tensor_tensor(out=act[:, :, :], in0=act[:, :, :],
                                    in1=cb[:, qi * nt:(qi + 1) * nt, :],
                                    op=mybir.AluOpType.add)
                    v.drain()
                # self-loop: g_sb already holds h*dinv (source side)
                v.tensor_tensor(out=act[:, :, :], in0=act[:, :, :],
                                in1=g_sb[:, :, :], op=mybir.AluOpType.add)
                v.drain()
                v.tensor_tensor(
                    out=act[:, :, :], in0=act[:, :, :],
                    in1=bass.AP(dinv_sb, 0,
                                [[dinv_sb.ap().ap[0][0], P], [1, nt], [0, F]]),
                    op=mybir.AluOpType.mult)
                v.drain()
                if l0 < 2:
                    bias = b1b_sb if l0 == 0 else b2b_sb
                    v.tensor_tensor(
                        out=act[:, :, :], in0=act[:, :, :],
                        in1=bass.AP(bias, 0,
                                    [[bias.ap().ap[0][0], P], [0, nt], [1, F]]),
                        op=mybir.AluOpType.add)
                    v.drain()
                    v.tensor_scalar_max(out=act[:, :, :], in0=act[:, :, :],
                                        scalar1=0.0)
                    v.drain()
                    v.tensor_copy(out=act16[:, :, :], in_=act[:, :, :]).then_inc(s_pp, 1)
                    v.drain()
                else:
                    v.tensor_tensor(
                        out=rt[:, 0:nt, :], in0=act[:, :, :],
                        in1=bass.AP(w3b_sb, 0,
                                    [[w3b_sb.ap().ap[0][0], P], [0, nt], [1, F]]),
                        op=mybir.AluOpType.mult)
                    v.drain()
                    v.tensor_reduce(out=outc[:, :], in_=rt[:, 0:nt, :],
                                    axis=mybir.AxisListType.X,
                                    op=mybir.AluOpType.add)
                    v.drain()
                    v.tensor_tensor(
                        out=outc[:, :], in0=outc[:, :],
                        in1=bass.AP(b3b_sb, 0,
                                    [[b3b_sb.ap().ap[0][0], P], [0, nt]]),
                        op=mybir.AluOpType.add).then_inc(s_pp, 1)

    nc.compile()
    return nc


def _emit_tree(v, bass, mybir, gb, rt, target, col0, t0, ntl, s, P, F):
    """Sum s slot-columns per tile (f16 in gb) into target[:, t0:t0+ntl, :] f32."""
    _eng = v
    def tensor_tensor_drained(**kw):
        r = _eng.tensor_tensor(**kw)
        _eng.drain()
        return r
    class _V:
        tensor_tensor = staticmethod(tensor_tensor_drained)
    v = _V()
    gp0 = gb.ap().ap[0][0]
    rp0 = rt.ap().ap[0][0]
    tp0 = target.ap().ap[0][0]
    add = mybir.AluOpType.add

    def gbap(coff, tilestride, cols):
        return bass.AP(gb, (col0 + coff) * F,
                       [[gp0, P], [tilestride * F, ntl], [F, cols], [1, F]])

    def rtap(coff, tilestride, cols):
        return bass.AP(rt, coff * F,
                       [[rp0, P], [tilestride * F, ntl], [F, cols], [1, F]])

    def tgap():
        return bass.AP(target, t0 * F, [[tp0, P], [F, ntl], [1, F]])

    if s == 2:
        v.tensor_tensor(out=tgap(), in0=gbap(0, s, 1), in1=gbap(1, s, 1), op=add)
        return
    h = s // 2
    odd = s % 2
    rstride = h
    # level 0: gb -> rt
    if h == 1:
        v.tensor_tensor(out=tgap(), in0=gbap(0, s, 1), in1=gbap(1, s, 1), op=add)
        if odd:
            v.tensor_tensor(out=tgap(), in0=tgap(), in1=gbap(2, s, 1), op=add)
        return
    v.tensor_tensor(out=rtap(0, rstride, h), in0=gbap(0, s, h),
                    in1=gbap(h, s, h), op=add)
    if odd:
        v.tensor_tensor(out=rtap(0, rstride, 1), in0=rtap(0, rstride, 1),
                        in1=gbap(2 * h, s, 1), op=add)
    k = h
    while k > 1:
        h2 = k // 2
        odd2 = k % 2
        if h2 == 1:
            v.tensor_tensor(out=tgap(), in0=rtap(0, rstride, 1),
                            in1=rtap(1, rstride, 1), op=add)
            if odd2:
                v.tensor_tensor(out=tgap(), in0=tgap(),
                                in1=rtap(2, rstride, 1), op=add)
            return
        v.tensor_tensor(out=rtap(0, rstride, h2), in0=rtap(0, rstride, h2),
                        in1=rtap(h2, rstride, h2), op=add)
        if odd2:
            v.tensor_tensor(out=rtap(0, rstride, 1), in0=rtap(0, rstride, 1),
                            in1=rtap(2 * h2, rstride, 1), op=add)
        k = h2


# ======================= top-level kernel =======================

def _build_inputs(st, x, W1, b1, W2, b2, W3, b3):
    in_maps = []
    eye = np.eye(P, dtype=np.float16)
    z4 = np.zeros((4, F), dtype=np.float16)
    for c in range(st.ncores):
        order0 = st.s0_orders[c]
        xs = np.zeros((st.nloc, FIN), dtype=np.float16)
        # row r = p*nt + t holds x of dst at stream-0 position t*128+p;
        # xt columns are TILE-major (t*128+p) for contiguous matmul lhsT.
        xloc = x[c * st.own + order0].astype(np.float16)    # [own, FIN] in s0 order
        xt = np.zeros((FIN, st.nloc), dtype=np.float16)
        xt[:, :st.own] = xloc.T                              # col j = s0 position j
        in_maps.append(dict(
            xt=np.ascontiguousarray(xt),
            w1=W1.astype(np.float16),
            w2=W2.astype(np.float16),
            w3b=np.ascontiguousarray(np.tile(W3[:, 0][None, :], (P, 1)).astype(np.float32)),
            b1b=np.ascontiguousarray(np.tile(b1[None, :], (P, 1)).astype(np.float32)),
            b2b=np.ascontiguousarray(np.tile(b2[None, :], (P, 1)).astype(np.float32)),
            b3b=np.full((P, 1), float(b3[0]), dtype=np.float32),
            dinv=st.dinv_pm[c],
            iden=eye,
            gidx=st.gidx[c],
            cidx=st.cidx[c],
            z4=z4,
        ))
    return in_maps


def kernel(x, edge_index, W1, b1, W2, b2, W3, b3):
    from concourse.bass_utils import run_bass_kernel_spmd
    x = np.asarray(x)
    st = prep(np.asarray(edge_index))
    nc = build_nc(st)
    in_maps = _build_inputs(st, x, np.asarray(W1), np.asarray(b1),
                            np.asarray(W2), np.asarray(b2),
                            np.asarray(W3), np.asarray(b3))
    res = run_bass_kernel_spmd(nc, in_maps, list(range(st.ncores)))
    out = np.zeros(st.n_nodes, dtype=np.float32)
    for c in range(st.ncores):
        oc = res.results[c]["out"]          # [P, nt]
        order0 = st.s0_orders[c]
        i = np.arange(st.own)
        out[c * st.own + order0] = oc[i % P, i // P]
    return out

